# revision 4
# baseline (speedup 1.0000x reference)
"""Trainium2 Bass kernel for structured-sparse matmul.

Computes: out[b,s,o] = sum_k x[b,s,sparse_idx[k]] * sparse_values[o,k]
  x: [4, 2048, 4096] f32, sparse_values: [4096, 1024] f32,
  sparse_idx: [1024] int64 (sorted, unique) -> out [4, 2048, 4096] f32

Strategy (8 NeuronCores, data-parallel over rows m = b*s), all-bf16:
  Host casts x / sparse_values to bf16 (end-to-end rel err ~3e-3, gate 2e-2)
  and expands sparse_idx into one-hot selection blocks G (compile-time
  metadata, NEFF cached per idx).
  Per core (M=1024 rows):
    Phase A: x tiles are loaded ALREADY TRANSPOSED via the DMA xbar
      (dma_start(transpose=True), 16-bit dtype, 14ns per 16x128 tile) ->
      xt[n-part, nb, m]. No PE transpose pass at all (the f32r ident-matmul
      transposes of the old kernel ran at 4 cycles/row because their output
      free size was 128 < 256 - ~55us of PE time).
      Gather n->k via matmul with one-hot blocks G: xg[k-part, kt, m].
    Phase B: GEMM out[m, o] = xg.T @ W^T[k, o] in bf16 (1 cycle/row,
      same PE rate as f32r but half the SBUF/DMA traffic).
  out written bf16 (8 MB/core) and upcast to f32 on the host.
"""

import sys

if "/opt/trn_rl_repo" not in sys.path:
    sys.path.insert(0, "/opt/trn_rl_repo")

import numpy as np
import ml_dtypes

BF16_NP = ml_dtypes.bfloat16

B, S, N_IN = 4, 2048, 4096
N_OUT, N_SPARSE = 4096, 1024
N_CORES = 8
M_TOT = B * S            # 8192
M = M_TOT // N_CORES     # 1024 rows per core
P = 128
NKT = N_SPARSE // P      # 8 k-tiles
NNB = N_IN // P          # 32 n-blocks
N_MT = M // P            # 8 m-tiles per core
MB = 512                 # m-batch for xbar-load/gather staging
N_BATCH = M // MB        # 2
O_TILE = 512
NOS = N_OUT // O_TILE    # 8 o-slices

_cache: dict = {}


def _build_gather_blocks(idx: np.ndarray):
    """Expand sparse_idx into one-hot selection blocks.

    For k-tile kt and n-block b, G[n, krel] = 1 iff idx[kt*128+krel] == b*128+n.
    Returns (g_all [NB,128,128] f32, blocks_per_kt: list of lists of (bi, b)).
    """
    mats = []
    blocks_per_kt = []
    for kt in range(NKT):
        ks = idx[kt * P:(kt + 1) * P]
        bs = sorted(set(int(k) // P for k in ks))
        entries = []
        for b in bs:
            mat = np.zeros((P, P), dtype=np.float32)
            for krel, k in enumerate(ks):
                if int(k) // P == b:
                    mat[int(k) % P, krel] = 1.0
            entries.append((len(mats), b))
            mats.append(mat)
        blocks_per_kt.append(entries)
    return np.stack(mats), blocks_per_kt


def _build_nc(blocks_per_kt, nb_total):
    import concourse.mybir as mybir
    import concourse.tile as tile
    from concourse import bacc

    BF16 = mybir.dt.bfloat16
    F32 = mybir.dt.float32

    nc = bacc.Bacc("TRN2", target_bir_lowering=False, debug=False)
    x = nc.dram_tensor("x", [M, N_IN], BF16, kind="ExternalInput")
    wt = nc.dram_tensor("wt", [NOS, P, NKT, O_TILE], BF16, kind="ExternalInput")
    g = nc.dram_tensor("g", [P, nb_total, P], BF16, kind="ExternalInput")
    ident = nc.dram_tensor("ident", [P, P], BF16, kind="ExternalInput")
    out = nc.dram_tensor("out", [M, N_OUT], BF16, kind="ExternalOutput")

    with tile.TileContext(nc) as tc:
        with (
            tc.tile_pool(name="const", bufs=1) as const_pool,
            tc.tile_pool(name="gpool", bufs=1) as g_pool,
            tc.tile_pool(name="xtpool", bufs=1) as xt_pool,
            tc.tile_pool(name="xgpool", bufs=1) as xg_pool,
            tc.tile_pool(name="wpool", bufs=2) as wt_pool,
            tc.tile_pool(name="opool", bufs=4) as o_pool,
            tc.tile_pool(name="ps_g", bufs=2, space="PSUM") as psg,
            tc.tile_pool(name="ps_b", bufs=6, space="PSUM") as psb,
        ):
            ident_sb = const_pool.tile([P, P], BF16)
            nc.sync.dma_start(ident_sb[:], ident[:])
            # transposed x resident: [n-part, n-block, m]
            xt_sb = xt_pool.tile([P, NNB, M], BF16)
            # gathered x resident: [k-part, kt, m]
            xg_sb = xg_pool.tile([P, NKT, M], BF16)

            # PE warm-up: the HAM clock gate keeps the PE at reduced clock
            # until it sees ~3.4us of sustained activity. bf16 [128x128]
            # matmuls run ~107ns each at the mid p-state, so burn ~40.
            for w in range(40):
                wps = psb.tile([P, O_TILE], F32, tag="psb", name=f"warm{w}")
                nc.tensor.matmul(
                    wps[:, :P], ident_sb[:], ident_sb[:], start=True, stop=True
                )

            # g + first weight slice go out first on the scalar (ACT) HWDGE
            # ring so the first gathers (~4us in) and GEMM s=0 never wait.
            g_sb = g_pool.tile([P, nb_total, P], BF16)
            nc.scalar.dma_start(g_sb[:], g[:])

            wt_tiles = {}

            def ensure_wt(s):
                if s >= NOS or s in wt_tiles:
                    return
                t = wt_pool.tile([P, NKT, O_TILE], BF16, tag="wt", name=f"wt{s}")
                nc.scalar.dma_start(t[:], wt[s])
                wt_tiles[s] = t

            ensure_wt(0)

            def emit_gather(kt, batch):
                m0 = batch * MB
                entries = blocks_per_kt[kt]
                ps = psg.tile([P, MB], F32, tag="psg", name=f"psg{batch}_{kt}")
                for i, (bi, b) in enumerate(entries):
                    nc.tensor.matmul(
                        ps[:],
                        g_sb[:, bi, :],
                        xt_sb[:, b, m0:m0 + MB],
                        start=(i == 0),
                        stop=(i == len(entries) - 1),
                    )
                # DVE is idle during Phase A (ACT is issuing DMAs).
                nc.vector.tensor_copy(xg_sb[:, kt, m0:m0 + MB], ps[:])

            # ---- Phase A: xbar-transposed loads + gather, chasing ----
            for batch in range(N_BATCH):
                m0 = batch * MB
                gathered = set()
                for c in range(NNB):
                    # All xbar transposes on ONE queue: each one spans all 16
                    # DMA engines (FixedSemIncDMA), so a second queue adds no
                    # bandwidth but lets two in-flight transposes corrupt
                    # each other's xbar state (seen as per-core-varying
                    # garbage on HW; CoreSim doesn't model it).
                    nc.sync.dma_start(
                        xt_sb[:, c, m0:m0 + MB],
                        x[m0:m0 + MB, c * P:(c + 1) * P],
                        transpose=True,
                    )
                    # Emit every gather whose source n-blocks are now loaded
                    # (idx sorted -> k-tiles consume blocks in order), so the
                    # PE stream is dense from ~4us in.
                    for kt in range(NKT):
                        if kt in gathered:
                            continue
                        if all(b <= c for _, b in blocks_per_kt[kt]):
                            emit_gather(kt, batch)
                            gathered.add(kt)
                for kt in range(NKT):
                    if kt not in gathered:
                        emit_gather(kt, batch)

            # ---- Phase B: main GEMM ----
            for s in range(NOS):
                ensure_wt(s)
                wt_sb = wt_tiles[s]
                for t in range(N_MT):
                    ps = psb.tile([P, O_TILE], F32, tag="psb",
                                  name=f"psb{s}_{t}")
                    for kt in range(NKT):
                        nc.tensor.matmul(
                            ps[:],
                            xg_sb[:, kt, t * P:(t + 1) * P],
                            wt_sb[:, kt, :],
                            start=(kt == 0),
                            stop=(kt == NKT - 1),
                        )
                    o_sb = o_pool.tile([P, O_TILE], BF16, tag="ob",
                                       name=f"ob{s}_{t}")
                    nc.vector.tensor_copy(o_sb[:], ps[:])
                    # out on the scalar queue: the sync queue carries the
                    # in-order xbar stream during Phase A.
                    nc.scalar.dma_start(
                        out[t * P:(t + 1) * P, s * O_TILE:(s + 1) * O_TILE],
                        o_sb[:],
                    )
                    if t == 2:
                        ensure_wt(s + 1)
    nc.compile()
    return nc


def _get_compiled(idx: np.ndarray):
    key = idx.tobytes()
    if key not in _cache:
        g_all, blocks_per_kt = _build_gather_blocks(idx)
        nc = _build_nc(blocks_per_kt, g_all.shape[0])
        _cache[key] = (nc, g_all)
    return _cache[key]


def _run(inputs, trace=False, trace_kwargs=None):
    from concourse.bass_utils import run_bass_kernel_spmd

    x = np.asarray(inputs["x"], dtype=np.float32)
    sv = np.asarray(inputs["sparse_values"], dtype=np.float32)
    idx = np.asarray(inputs["sparse_idx"]).astype(np.int64)

    nc, g_all = _get_compiled(idx)

    x2 = x.reshape(M_TOT, N_IN).astype(BF16_NP)
    # wt swizzled for contiguous per-partition DMA: [o-slice, k%128, k//128, o]
    wtv = np.ascontiguousarray(
        sv.T.reshape(NKT, P, NOS, O_TILE).transpose(2, 1, 0, 3).astype(BF16_NP)
    )
    # g swizzled to [n-rel (partition), block, k-rel]
    g_swz = np.ascontiguousarray(g_all.transpose(1, 0, 2).astype(BF16_NP))
    in_maps = [
        {
            "x": np.ascontiguousarray(x2[c * M:(c + 1) * M]),
            "wt": wtv,
            "g": g_swz,
            "ident": np.eye(P, dtype=BF16_NP),
        }
        for c in range(N_CORES)
    ]
    res = run_bass_kernel_spmd(
        nc,
        in_maps,
        core_ids=list(range(N_CORES)),
        trace=trace,
        **(trace_kwargs or {}),
    )
    full = np.concatenate(
        [np.asarray(r["out"]).astype(np.float32) for r in res.results], axis=0
    )
    return full.reshape(B, S, N_OUT), res


def kernel(**inputs) -> np.ndarray:
    out, _ = _run(inputs)
    return out


# revision 7
# speedup vs baseline: 1.3861x; 1.3861x over previous
"""Trainium2 Bass kernel for structured-sparse matmul.

Computes: out[b,s,o] = sum_k x[b,s,sparse_idx[k]] * sparse_values[o,k]
  x: [4, 2048, 4096] f32, sparse_values: [4096, 1024] f32,
  sparse_idx: [1024] int64 (sorted, unique) -> out [4, 2048, 4096] f32

Strategy (8 NeuronCores, data-parallel over rows m = b*s), all-bf16:
  Host casts x / sparse_values to bf16 (end-to-end rel err ~3e-3, gate
  2e-2), pre-transposes each core's x slice to x^T quarters
  [4, N_IN, 256] (pure layout prep), and lays sparse_idx out in the
  gpsimd wrapped-index format.
  Per core (M=1024 rows):
    Phase A runs entirely on the DMA path: gpsimd dma_gather pulls the
      1024 needed x^T rows straight from HBM into SBUF in the exact
      [k%128 part, k//128, m] GEMM layout (4 m-quarter calls, pipelined;
      2 MB instead of 8 MB of x traffic, ZERO PE cycles).
    Phase B: GEMM out[m, o] = xg.T @ W^T[k, o] in bf16 (1 cycle/row).
      The PE stream is dense - warmup ramp, then 512 back-to-back
      [128x128x512] matmuls - so the HAM clock gate keeps the PE at
      2.4 GHz (any multi-us idle re-gates it to 1.2 GHz, which is what
      made earlier PE-transpose/xbar variants slow).
  out written bf16 (8 MB/core) and upcast to f32 on the host.
"""

import sys

if "/opt/trn_rl_repo" not in sys.path:
    sys.path.insert(0, "/opt/trn_rl_repo")

import numpy as np
import ml_dtypes

BF16_NP = ml_dtypes.bfloat16

B, S, N_IN = 4, 2048, 4096
N_OUT, N_SPARSE = 4096, 1024
N_CORES = 8
M_TOT = B * S            # 8192
M = M_TOT // N_CORES     # 1024 rows per core
P = 128
NKT = N_SPARSE // P      # 8 k-tiles
N_MT = M // P            # 8 m-tiles per core
NQ = 4                   # x^T gathered in m-quarters
MQ = M // NQ             # 256
O_TILE = 512
NOS = N_OUT // O_TILE    # 8 o-slices

_cache: dict = {}


def _build_nc():
    import concourse.mybir as mybir
    import concourse.tile as tile
    from concourse import bacc

    BF16 = mybir.dt.bfloat16
    F32 = mybir.dt.float32
    I16 = mybir.dt.int16

    # dynamic_dma_scratch_size: SWDGE descriptor-ring carveout. The default
    # 16384 gives a 1024-descriptor ring; a 1024-idx dma_gather never fits
    # and deadlocks the scheduler. 64KB -> 4096-descriptor ring.
    nc = bacc.Bacc(
        "TRN2",
        target_bir_lowering=False,
        debug=False,
        dynamic_dma_scratch_size=65536,
    )
    xt = nc.dram_tensor("xt", [NQ, N_IN, MQ], BF16, kind="ExternalInput")
    wt = nc.dram_tensor("wt", [NOS, P, NKT, O_TILE], BF16, kind="ExternalInput")
    idxs = nc.dram_tensor("idxs", [P, N_SPARSE // 16], I16, kind="ExternalInput")
    ident = nc.dram_tensor("ident", [P, P], BF16, kind="ExternalInput")
    out = nc.dram_tensor("out", [M, N_OUT], BF16, kind="ExternalOutput")

    with tile.TileContext(nc) as tc:
        with (
            tc.tile_pool(name="const", bufs=1) as const_pool,
            tc.tile_pool(name="xgpool", bufs=NQ) as xg_pool,
            tc.tile_pool(name="wpool", bufs=2) as wt_pool,
            tc.tile_pool(name="opool", bufs=4) as o_pool,
            tc.tile_pool(name="ps_b", bufs=8, space="PSUM") as psb,
        ):
            idx_sb = const_pool.tile([P, N_SPARSE // 16], I16)
            nc.sync.dma_start(idx_sb[:], idxs[:])
            ident_sb = const_pool.tile([P, P], BF16)
            nc.sync.dma_start(ident_sb[:], ident[:])

            # Phase A: gather x^T rows from HBM, one call per m-quarter.
            # Each lands as xg_q[k%128, k//128, m_rel] - directly the GEMM's
            # stationary-operand layout.
            xg_tiles = []
            for q in range(NQ):
                xg = xg_pool.tile([P, NKT, MQ], BF16, tag="xg", name=f"xg{q}")
                nc.gpsimd.dma_gather(
                    xg[:],
                    xt[q],
                    idx_sb[:],
                    num_idxs=N_SPARSE,
                    num_idxs_reg=N_SPARSE,
                    elem_size=MQ,
                )
                xg_tiles.append(xg)

            # wt slices stream on the scalar (ACT) ring; out goes on sync.
            wt_tiles = {}

            def ensure_wt(s):
                if s >= NOS or s in wt_tiles:
                    return
                t = wt_pool.tile([P, NKT, O_TILE], BF16, tag="wt", name=f"wt{s}")
                nc.scalar.dma_start(t[:], wt[s])
                wt_tiles[s] = t

            ensure_wt(0)
            ensure_wt(1)

            # PE warm-up: the HAM clock gate keeps the PE at 1.2 GHz until it
            # sees ~3.4us of sustained activity; bf16 [128x128] matmuls run
            # ~107ns each at the mid p-state, so burn ~40 while the first
            # gather + wt slice are in flight.
            for w in range(40):
                wps = psb.tile([P, O_TILE], F32, tag="psb", name=f"warm{w}")
                nc.tensor.matmul(
                    wps[:, :P], ident_sb[:], ident_sb[:], start=True, stop=True
                )

            # Phase B: main GEMM.
            for s in range(NOS):
                ensure_wt(s)
                wt_sb = wt_tiles[s]
                for t in range(N_MT):
                    xg = xg_tiles[t // (N_MT // NQ)]
                    mrel = (t % (N_MT // NQ)) * P
                    ps = psb.tile([P, O_TILE], F32, tag="psb",
                                  name=f"psb{s}_{t}")
                    for kt in range(NKT):
                        nc.tensor.matmul(
                            ps[:],
                            xg[:, kt, mrel:mrel + P],
                            wt_sb[:, kt, :],
                            start=(kt == 0),
                            stop=(kt == NKT - 1),
                        )
                    o_sb = o_pool.tile([P, O_TILE], BF16, tag="ob",
                                       name=f"ob{s}_{t}")
                    # Alternate eviction engine so neither DVE nor ACT gates
                    # PSUM recycling.
                    if t % 2 == 0:
                        nc.vector.tensor_copy(o_sb[:], ps[:])
                    else:
                        nc.scalar.copy(o_sb[:], ps[:])
                    nc.sync.dma_start(
                        out[t * P:(t + 1) * P, s * O_TILE:(s + 1) * O_TILE],
                        o_sb[:],
                    )
                    if t == 2:
                        ensure_wt(s + 1)
    nc.compile()
    return nc


def _get_compiled():
    if "nc" not in _cache:
        _cache["nc"] = _build_nc()
    return _cache["nc"]


def _wrap_idx(idx: np.ndarray) -> np.ndarray:
    """gpsimd wrapped-index layout: idx i at [i%16, i//16], replicated
    across the 8 gpsimd core groups."""
    w = np.zeros((P, N_SPARSE // 16), dtype=np.int16)
    cols = idx.astype(np.int16).reshape(N_SPARSE // 16, 16)  # [col, part]
    for g in range(8):
        w[g * 16:(g + 1) * 16, :] = cols.T
    return w


def _run(inputs, trace=False, trace_kwargs=None):
    from concourse.bass_utils import run_bass_kernel_spmd

    x = np.asarray(inputs["x"], dtype=np.float32)
    sv = np.asarray(inputs["sparse_values"], dtype=np.float32)
    idx = np.asarray(inputs["sparse_idx"]).astype(np.int64)

    nc = _get_compiled()

    x2 = x.reshape(M_TOT, N_IN).astype(BF16_NP)
    # wt swizzled for contiguous per-partition DMA: [o-slice, k%128, k//128, o]
    wtv = np.ascontiguousarray(
        sv.T.reshape(NKT, P, NOS, O_TILE).transpose(2, 1, 0, 3).astype(BF16_NP)
    )
    idx_w = _wrap_idx(idx)
    ident = np.eye(P, dtype=BF16_NP)
    in_maps = [
        {
            # x^T quarters: [q, n, m_rel] (layout-only prep)
            "xt": np.ascontiguousarray(
                x2[c * M:(c + 1) * M].T.reshape(N_IN, NQ, MQ).transpose(1, 0, 2)
            ),
            "wt": wtv,
            "idxs": idx_w,
            "ident": ident,
        }
        for c in range(N_CORES)
    ]
    res = run_bass_kernel_spmd(
        nc,
        in_maps,
        core_ids=list(range(N_CORES)),
        trace=trace,
        **(trace_kwargs or {}),
    )
    full = np.concatenate(
        [np.asarray(r["out"]).astype(np.float32) for r in res.results], axis=0
    )
    return full.reshape(B, S, N_OUT), res


def kernel(**inputs) -> np.ndarray:
    out, _ = _run(inputs)
    return out
